# revision 50
# baseline (speedup 1.0000x reference)
"""Axial (frame-local) attention kernel for Trainium2, 8-core data-parallel.

Problem: x[4, 8192, 512] -> qkv proj -> per-(batch, head, frame) attention over
256-token frames (f=32 frames of 256 tokens in an 8192 sequence) -> out proj.

Sharding: pure data-parallel over (batch, half-sequence): core c handles
batch c//2, tokens (c%2)*4096 .. +4096 (16 whole frames). No collectives.

Per-core pipeline (chunks of 512 tokens):
  - load x chunk, PE-transpose into xT [dim, tok] (feature-major)
  - qT,kT = (w_qkv block)^T-matmul in [feat, tok] layout; v natural [tok, feat]
  - per (frame, head): sim^T = k q^T on PE -> exp on ScalarE (no max-subtract;
    logits are O(6) so fp32 exp is safe) -> ov = [v|1]^T p~ on PE produces both
    the unnormalized attention output AND the softmax denominator Z (row 64)
  - normalize: 1/Z = exp(-ln Z) on ScalarE (DVE reciprocal is 8 cyc/elem —
    too slow), GPSIMD partition-broadcast, one DVE multiply
  - output projection from the transposed layout + bias, DMA out

Matmul operands use float32r (single-pass fp32, ~tf32 precision, 2x faster
than fp32's LOW_HIGH two-pass mode). PSUM accumulation stays fp32.
"""

import sys
import types

import numpy as np

import concourse.tile as tile
from concourse import bacc, mybir
from concourse.bass import ts
from concourse.bass_utils import run_bass_kernel_spmd
from concourse.masks import make_identity

F32 = mybir.dt.float32
F32R = mybir.dt.float32r
AF = mybir.ActivationFunctionType
ALU = mybir.AluOpType

# Model dims (hardcoded per problem spec)
B, SEQ, D = 4, 8192, 512
HEADS, DH = 8, 64
INNER = HEADS * DH  # 512
SCALE = DH ** -0.5
FRAME = 256  # n_sp = seq // f = 8192 // 32
N_CORES = 8
T = (B * SEQ) // N_CORES  # 4096 tokens per core
CHUNK = 512  # tokens per inner iteration
NCH = T // CHUNK  # 8
FPC = CHUNK // FRAME  # frames per chunk = 2
SPC = CHUNK // 128  # 128-token subtiles per chunk = 4

# matmul operand dtype: F32R (single-pass, ~tf32) or F32 (two-pass, exact)
MM_DT = F32R
# attention (sim / attn@v) operand dtype: BF16 enables fast-weight-load and
# LDWEIGHTS pull-ahead (f32r weight loads can't hide behind matmuls)
AT_DT = F32R

def _install_ntff_hook():
    """The trimmed container's antenv lacks axon_hooks; inject it so
    run_bass_kernel_spmd(trace=True) can capture NTFF profiles."""
    if "antenv.axon_hooks" in sys.modules:
        return
    try:
        from trn_agent_boot.trn_boot import _ntff_profile_via_ctypes

        hook = _ntff_profile_via_ctypes("/opt/axon/libaxon_pjrt.so")
    except Exception:
        return
    mod = types.ModuleType("antenv.axon_hooks")
    mod._hook = hook
    mod.get_axon_ntff_profile_hook = lambda: mod._hook
    mod.set_axon_ntff_profile_hook = lambda h: setattr(mod, "_hook", h)
    sys.modules["antenv.axon_hooks"] = mod


def _pin_act_tables():
    """Exp and Ln both live in the natural_log_exp_and_others table set, but
    the table-load chooser maps each function to the first set containing it,
    so alternating Exp/Ln activations reload tables (~1.3us each) every head
    pair. Restrict Exp/Ln to the combined set in the cached table map so one
    load covers the whole kernel."""
    from concourse.hw_specs import get_activation_tables

    tabs = get_activation_tables(_pin_act_tables.arch)
    keep = "natural_log_exp_and_others"
    if keep not in tabs:
        return
    for name, fns in tabs.items():
        if name != keep:
            fns.discard(AF.Exp)
            fns.discard(AF.Ln)


def _build_body(nc, tc, ctx, x_ap, wqkv_ap, wout_ap, bout_ap, out_ap, n_chunks=NCH):
    mm_dt = MM_DT
    at_dt = AT_DT
    pconst = ctx.enter_context(tc.tile_pool(name="const", bufs=1))
    px = ctx.enter_context(tc.tile_pool(name="x", bufs=2))
    pxt = ctx.enter_context(tc.tile_pool(name="xt", bufs=8))
    pqk = ctx.enter_context(tc.tile_pool(name="qk", bufs=16))
    pvx = ctx.enter_context(tc.tile_pool(name="vx", bufs=6))
    ppt = ctx.enter_context(tc.tile_pool(name="pt", bufs=4))
    prz = ctx.enter_context(tc.tile_pool(name="rz", bufs=3))
    prb = ctx.enter_context(tc.tile_pool(name="rb", bufs=3))
    pov = ctx.enter_context(tc.tile_pool(name="ovs", bufs=3))
    pot = ctx.enter_context(tc.tile_pool(name="ot", bufs=6))
    py = ctx.enter_context(tc.tile_pool(name="y", bufs=3))
    pmm = ctx.enter_context(tc.tile_pool(name="mm", bufs=2, space="PSUM"))
    psim = ctx.enter_context(tc.tile_pool(name="sim", bufs=2, space="PSUM"))
    povp = ctx.enter_context(tc.tile_pool(name="ovp", bufs=2, space="PSUM"))

    # Constants. The weight DMAs are emitted by load_consts() AFTER the
    # first two x-chunk loads so chunk 0's transposes aren't queued behind
    # 4MB of weights; spread across three DMA queues.
    ident = pconst.tile([128, 128], F32, tag="ident")
    make_identity(nc, ident[:])
    w_kts = [
        pconst.tile([128, 3 * INNER], mm_dt, tag=f"wqkv{kt}", name=f"wqkv{kt}")
        for kt in range(4)
    ]
    wo_sb = pconst.tile([128, 4, D], mm_dt, tag="wout")
    b1 = pconst.tile([1, D], F32, tag="b1")
    bb = pconst.tile([128, D], F32, tag="bb")

    def load_consts():
        qeng = [nc.scalar, nc.gpsimd, nc.scalar, nc.sync]
        for kt in range(4):
            qeng[kt].dma_start(
                w_kts[kt][:], wqkv_ap.bitcast(mm_dt)[kt * 128 : (kt + 1) * 128, :]
            )
        nc.scalar.dma_start(
            wo_sb[:], wout_ap.bitcast(mm_dt).rearrange("(kt p) e -> p kt e", p=128)
        )
        nc.scalar.dma_start(b1[:], bout_ap.rearrange("(a d) -> a d", a=1))
        nc.gpsimd.partition_broadcast(bb[:], b1[:])

    def ld(ci):
        tb = ci * CHUNK

        # ---- load x chunk [128, subtile, D] (token-major) ----
        x_t = px.tile([128, SPC, D], F32, tag="x")
        for t in range(SPC):
            nc.sync.dma_start(
                x_t[:, t, :], x_ap[tb + t * 128 : tb + (t + 1) * 128, :]
            )

        # ---- transpose to xT: 4 tiles [128 dim, CHUNK tok] ----
        xts = []
        for db in range(4):
            xtp = pmm.tile([128, CHUNK], F32, tag="mm")
            for t in range(SPC):
                nc.tensor.transpose(
                    xtp[:, ts(t, 128)], x_t[:, t, ts(db, 128)], ident[:]
                )
            xt = pxt.tile([128, CHUNK], mm_dt, tag="xt", bufs=10)
            nc.any.tensor_copy(xt[:], xtp[:])
            xts.append(xt)
        return xts

    def qv(ci, xts):
        # ---- qT, kT in [feat, tok] layout: 8 ptiles of 128 feats ----
        qkd = {}
        qkod = {}
        # k-ptiles for quad 0 first: attention's first sim matmuls need
        # ptiles (4,5,0,1); emitting in that order shortens the wait
        for p in (4, 5, 0, 1, 6, 7, 2, 3):
            ps = pmm.tile([128, CHUNK], F32, tag="mm")
            for kt in range(4):
                nc.tensor.matmul(
                    ps[:],
                    w_kts[kt][:, ts(p, 128)],
                    xts[kt][:],
                    start=(kt == 0),
                    stop=(kt == 3),
                )
            qs = pqk.tile([128, CHUNK], at_dt, tag="qk", bufs=10)
            nc.vector.tensor_copy(qs[:], ps[:])
            qkd[p] = qs
            # odd heads live at partitions 64-127; matmul operands must sit
            # at base partition 0 (tile_position row 64 faults on this
            # runtime), so shift them down with SBUF->SBUF DMA right after
            # the cast (DMA is address-based)
            qo = pqk.tile([64, CHUNK], at_dt, tag="qko", name=f"qko{ci}_{p}", bufs=9)
            nc.sync.dma_start(qo[:], qs[64:128, :])
            qkod[p] = qo

        qkts = [qkd[p] for p in range(8)]
        qkos = [qkod[p] for p in range(8)]

        # ---- v natural [tok, feat] + ones column -> vext [128, h, 65] ----
        vexts = []
        for t in range(SPC):
            ps = pmm.tile([128, INNER], F32, tag="mm")
            for kt in range(4):
                nc.tensor.matmul(
                    ps[:],
                    xts[kt][:, ts(t, 128)],
                    w_kts[kt][:, 2 * INNER : 3 * INNER],
                    start=(kt == 0),
                    stop=(kt == 3),
                )
            vx = pvx.tile([128, HEADS, DH + 1], at_dt, tag="vx", bufs=5)
            nc.vector.memset(vx[:, :, DH : DH + 1].bitcast(F32), 1.0)
            nc.vector.tensor_copy(
                vx[:, :, 0:DH], ps[:].rearrange("p (h d) -> p h d", h=HEADS)
            )
            vexts.append(vx)

        return qkts, qkos, vexts

    def attn(ci, st):
        qkts, qkos, vexts = st
        # ---- attention, output written transposed into outT ptiles ----
        # Even heads (rows 0-63 of a ptile) write otls directly; odd heads
        # compute into base-0 tiles (oto) and are DMA-shifted to rows 64-127.
        otls = [
            pot.tile([128, CHUNK], mm_dt, tag="ot", name=f"ot{ci}_{i}")
            for i in range(4)
        ]
        otos = [
            pot.tile([64, CHUNK], mm_dt, tag="oto", name=f"oto{ci}_{i}")
            for i in range(4)
        ]
        half = []
        zst = prz.tile([97, 2 * FRAME], F32, tag="zst", name=f"zst{ci}_init")
        nc.vector.memset(zst[:], 1.0)
        for pr in range(4):  # head pairs (2*pr, 2*pr+1)
            # sim^T for both heads x both frames, per key-side 128-tok tile:
            # psum cols = (hp, fi) * FRAME
            pts = []
            for jt in range(2):
                sim = psim.tile([128, 4 * FRAME], F32, tag="sim")
                for hp in range(2):
                    h = 2 * pr + hp
                    if h % 2 == 0:
                        ck = qkts[4 + h // 2][0:64, :]
                        cq = qkts[h // 2][0:64, :]
                    else:
                        ck = qkos[4 + h // 2][:]
                        cq = qkos[h // 2][:]
                    for fi in range(FPC):
                        f0 = fi * FRAME
                        nc.tensor.matmul(
                            sim[:, ts(2 * hp + fi, FRAME)],
                            ck[:, f0 + jt * 128 : f0 + (jt + 1) * 128],
                            cq[:, f0 : f0 + FRAME],
                            start=True,
                            stop=True,
                        )
                pt = ppt.tile([128, 4 * FRAME], at_dt, tag="pt")
                nc.scalar.activation(pt[:], sim[:], AF.Exp, scale=SCALE)
                pts.append(pt)
            for fi in range(FPC):
                ovp = povp.tile([DH + 1, 2 * FRAME], F32, tag="ovp")
                for hp in range(2):
                    for jt in range(2):
                        nc.tensor.matmul(
                            ovp[:, ts(hp, FRAME)],
                            vexts[fi * 2 + jt][:, 2 * pr + hp, :],
                            pts[jt][:, ts(2 * hp + fi, FRAME)],
                            start=(jt == 0),
                            stop=(jt == 1),
                        )
                # unnormalized outputs (+ Z row) to SBUF; frees the PSUM tile
                ovs = pov.tile([DH + 1, 2 * FRAME], F32, tag="ovs", bufs=6)
                nc.vector.tensor_copy(ovs[:], ovp[:])
                # gather this pair's softmax denominators into the half-chunk
                # staging tile (DMA shifts partitions; DVE/ACT cannot)
                gi = 2 * (pr % 2) + fi
                nc.sync.dma_start(zst[32 * gi : 32 * gi + 1, :], ovs[DH : DH + 1, :])
                half.append((pr, fi, ovs))
            if pr % 2 == 1:
                # batch 1/Z = exp(-ln Z) for 4 (pair, frame) groups at once:
                # [4, 512] activations amortize the ~300-cycle ACT overhead
                lnst = prz.tile([97, 2 * FRAME], F32, tag="lnst")
                nc.scalar.activation(lnst[:], zst[:], AF.Ln)
                rzst = prz.tile([97, 2 * FRAME], F32, tag="rzst")
                nc.scalar.activation(rzst[:], lnst[:], AF.Exp, scale=-1.0)
                for bpr, bfi, ovs in half:
                    f0 = bfi * FRAME
                    gi = 2 * (bpr % 2) + bfi
                    rb = prb.tile([DH, 2 * FRAME], F32, tag="rb")
                    nc.gpsimd.partition_broadcast(rb[:], rzst[32 * gi : 32 * gi + 1, :])
                    for hp in range(2):
                        h = 2 * bpr + hp
                        dst = (
                            otls[h // 2][0:DH] if h % 2 == 0 else otos[h // 2][:]
                        )
                        nc.vector.tensor_mul(
                            dst[:, f0 : f0 + FRAME],
                            ovs[0:DH, ts(hp, FRAME)],
                            rb[:, ts(hp, FRAME)],
                        )
                    if bfi == FPC - 1:
                        # odd head of this pair is complete: shift its rows
                        # into the ptile so proj isn't gated on one big DMA
                        nc.sync.dma_start(otls[bpr][64:128, :], otos[bpr][:])
                half = []
                zst = prz.tile([97, 2 * FRAME], F32, tag="zst", name=f"zst{ci}_{pr}")
                nc.vector.memset(zst[:], 1.0)

        return otls

    def proj(ci, otls):
        tb = ci * CHUNK
        # ---- output projection + bias ----
        for s in range(SPC):
            ps = pmm.tile([128, D], F32, tag="mm")
            for p in range(4):
                nc.tensor.matmul(
                    ps[:],
                    otls[p][:, ts(s, 128)],
                    wo_sb[:, p, :],
                    start=(p == 0),
                    stop=(p == 3),
                )
            y = py.tile([128, D], F32, tag="y", bufs=3)
            nc.vector.scalar_tensor_tensor(
                y[:], ps[:], 1.0, bb[:], op0=ALU.mult, op1=ALU.add
            )
            nc.sync.dma_start(out_ap[tb + s * 128 : tb + (s + 1) * 128, :], y[:])

    # Software pipeline: emit the next chunk's transposes and qkv between
    # this chunk's attention and projection so PE has work while the
    # softmax-normalize chain (ACT->DVE->GPSIMD->DVE) drains; run the
    # transpose stage two chunks ahead so chunk 0's weight-load wait is
    # covered too.
    lds = {0: ld(0)}
    load_consts()
    if n_chunks > 1:
        lds[1] = ld(1)
    st = qv(0, lds.pop(0))
    for ci in range(n_chunks):
        otls = attn(ci, st)
        if ci + 1 < n_chunks:
            if ci + 2 < n_chunks:
                lds[ci + 2] = ld(ci + 2)
            st = qv(ci + 1, lds.pop(ci + 1))
        proj(ci, otls)


_CACHE = {}


def _get_nc(n_chunks=NCH):
    key = ("nc", n_chunks, str(MM_DT))
    if key in _CACHE:
        return _CACHE[key]
    from contextlib import ExitStack

    nc = bacc.Bacc("TRN2", target_bir_lowering=False, debug=False, num_devices=N_CORES)
    _pin_act_tables.arch = nc.m.arch
    _pin_act_tables()
    t_tok = n_chunks * CHUNK
    x_ap = nc.dram_tensor("x", [t_tok, D], F32, kind="ExternalInput").ap()
    wqkv_ap = nc.dram_tensor("w_qkv", [D, 3 * INNER], F32, kind="ExternalInput").ap()
    wout_ap = nc.dram_tensor("w_out", [INNER, D], F32, kind="ExternalInput").ap()
    bout_ap = nc.dram_tensor("b_out", [D], F32, kind="ExternalInput").ap()
    out_ap = nc.dram_tensor("out", [t_tok, D], F32, kind="ExternalOutput").ap()
    with tile.TileContext(nc) as tc:
        with ExitStack() as ctx:
            _build_body(
                nc, tc, ctx, x_ap, wqkv_ap, wout_ap, bout_ap, out_ap, n_chunks=n_chunks
            )
    nc.compile()
    _CACHE[key] = nc
    return nc


def _make_in_maps(x, w_qkv, w_out, b_out):
    x = np.ascontiguousarray(np.asarray(x, dtype=np.float32))
    w_qkv = np.ascontiguousarray(np.asarray(w_qkv, dtype=np.float32))
    w_out = np.ascontiguousarray(np.asarray(w_out, dtype=np.float32))
    b_out = np.ascontiguousarray(np.asarray(b_out, dtype=np.float32))
    assert x.shape == (B, SEQ, D), x.shape
    in_maps = []
    for c in range(N_CORES):
        b = c // 2
        t0 = (c % 2) * T
        in_maps.append(
            {
                "x": np.ascontiguousarray(x[b, t0 : t0 + T, :]),
                "w_qkv": w_qkv,
                "w_out": w_out,
                "b_out": b_out,
            }
        )
    return in_maps


def _assemble(results):
    out = np.empty((B, SEQ, D), dtype=np.float32)
    for c in range(N_CORES):
        b = c // 2
        t0 = (c % 2) * T
        out[b, t0 : t0 + T, :] = results[c]["out"]
    return out


def run(x, w_qkv, w_out, b_out, f=32, trace=False):
    assert int(f) == 32, f"kernel hardcoded for f=32, got {f}"
    _install_ntff_hook()
    nc = _get_nc()
    in_maps = _make_in_maps(x, w_qkv, w_out, b_out)
    res = run_bass_kernel_spmd(nc, in_maps, list(range(N_CORES)), trace=trace)
    return _assemble(res.results), res


def kernel(x, w_qkv, w_out, b_out, f=32):
    out, _ = run(x, w_qkv, w_out, b_out, f=f, trace=False)
    return out


# revision 52
# speedup vs baseline: 1.0341x; 1.0341x over previous
"""Axial (frame-local) attention kernel for Trainium2, 8-core data-parallel.

Problem: x[4, 8192, 512] -> qkv proj -> per-(batch, head, frame) attention over
256-token frames (f=32 frames of 256 tokens in an 8192 sequence) -> out proj.

Sharding: pure data-parallel over (batch, half-sequence): core c handles
batch c//2, tokens (c%2)*4096 .. +4096 (16 whole frames). No collectives.

Per-core pipeline (chunks of 512 tokens):
  - load x chunk, PE-transpose into xT [dim, tok] (feature-major)
  - qT,kT = (w_qkv block)^T-matmul in [feat, tok] layout; v natural [tok, feat]
  - per (frame, head): sim^T = k q^T on PE -> exp on ScalarE (no max-subtract;
    logits are O(6) so fp32 exp is safe) -> ov = [v|1]^T p~ on PE produces both
    the unnormalized attention output AND the softmax denominator Z (row 64)
  - normalize: 1/Z = exp(-ln Z) on ScalarE (DVE reciprocal is 8 cyc/elem —
    too slow), GPSIMD partition-broadcast, one DVE multiply
  - output projection from the transposed layout + bias, DMA out

Matmul operands use float32r (single-pass fp32, ~tf32 precision, 2x faster
than fp32's LOW_HIGH two-pass mode). PSUM accumulation stays fp32.
"""

import sys
import types

import numpy as np

import concourse.tile as tile
from concourse import bacc, mybir
from concourse.bass import ts
from concourse.bass_utils import run_bass_kernel_spmd
from concourse.masks import make_identity

F32 = mybir.dt.float32
F32R = mybir.dt.float32r
AF = mybir.ActivationFunctionType
ALU = mybir.AluOpType

# Model dims (hardcoded per problem spec)
B, SEQ, D = 4, 8192, 512
HEADS, DH = 8, 64
INNER = HEADS * DH  # 512
SCALE = DH ** -0.5
FRAME = 256  # n_sp = seq // f = 8192 // 32
N_CORES = 8
T = (B * SEQ) // N_CORES  # 4096 tokens per core
CHUNK = 512  # tokens per inner iteration
NCH = T // CHUNK  # 8
FPC = CHUNK // FRAME  # frames per chunk = 2
SPC = CHUNK // 128  # 128-token subtiles per chunk = 4

# matmul operand dtype: F32R (single-pass, ~tf32) or F32 (two-pass, exact)
MM_DT = F32R
# attention (sim / attn@v) operand dtype: BF16 enables fast-weight-load and
# LDWEIGHTS pull-ahead (f32r weight loads can't hide behind matmuls)
AT_DT = F32R

def _install_ntff_hook():
    """The trimmed container's antenv lacks axon_hooks; inject it so
    run_bass_kernel_spmd(trace=True) can capture NTFF profiles."""
    if "antenv.axon_hooks" in sys.modules:
        return
    try:
        from trn_agent_boot.trn_boot import _ntff_profile_via_ctypes

        hook = _ntff_profile_via_ctypes("/opt/axon/libaxon_pjrt.so")
    except Exception:
        return
    mod = types.ModuleType("antenv.axon_hooks")
    mod._hook = hook
    mod.get_axon_ntff_profile_hook = lambda: mod._hook
    mod.set_axon_ntff_profile_hook = lambda h: setattr(mod, "_hook", h)
    sys.modules["antenv.axon_hooks"] = mod


def _pin_act_tables():
    """Exp and Ln both live in the natural_log_exp_and_others table set, but
    the table-load chooser maps each function to the first set containing it,
    so alternating Exp/Ln activations reload tables (~1.3us each) every head
    pair. Restrict Exp/Ln to the combined set in the cached table map so one
    load covers the whole kernel."""
    from concourse.hw_specs import get_activation_tables

    tabs = get_activation_tables(_pin_act_tables.arch)
    keep = "natural_log_exp_and_others"
    if keep not in tabs:
        return
    for name, fns in tabs.items():
        if name != keep:
            fns.discard(AF.Exp)
            fns.discard(AF.Ln)


def _build_body(nc, tc, ctx, x_ap, wqkv_ap, wout_ap, bout_ap, out_ap, n_chunks=NCH):
    mm_dt = MM_DT
    at_dt = AT_DT
    pconst = ctx.enter_context(tc.tile_pool(name="const", bufs=1))
    px = ctx.enter_context(tc.tile_pool(name="x", bufs=2))
    pxt = ctx.enter_context(tc.tile_pool(name="xt", bufs=8))
    pqk = ctx.enter_context(tc.tile_pool(name="qk", bufs=16))
    pvx = ctx.enter_context(tc.tile_pool(name="vx", bufs=6))
    ppt = ctx.enter_context(tc.tile_pool(name="pt", bufs=4))
    prz = ctx.enter_context(tc.tile_pool(name="rz", bufs=2))
    prb = ctx.enter_context(tc.tile_pool(name="rb", bufs=3))
    pov = ctx.enter_context(tc.tile_pool(name="ovs", bufs=3))
    pot = ctx.enter_context(tc.tile_pool(name="ot", bufs=6))
    py = ctx.enter_context(tc.tile_pool(name="y", bufs=3))
    pmm = ctx.enter_context(tc.tile_pool(name="mm", bufs=2, space="PSUM"))
    psim = ctx.enter_context(tc.tile_pool(name="sim", bufs=2, space="PSUM"))
    povp = ctx.enter_context(tc.tile_pool(name="ovp", bufs=2, space="PSUM"))

    # Constants. The weight DMAs are emitted by load_consts() AFTER the
    # first two x-chunk loads so chunk 0's transposes aren't queued behind
    # 4MB of weights; spread across three DMA queues.
    ident = pconst.tile([128, 128], F32, tag="ident")
    make_identity(nc, ident[:])
    w_kts = [
        pconst.tile([128, 3 * INNER], mm_dt, tag=f"wqkv{kt}", name=f"wqkv{kt}")
        for kt in range(4)
    ]
    wo_sb = pconst.tile([128, 4, D], mm_dt, tag="wout")
    b1 = pconst.tile([1, D], F32, tag="b1")
    bb = pconst.tile([128, D], F32, tag="bb")

    def load_consts():
        qeng = [nc.scalar, nc.gpsimd, nc.scalar, nc.sync]
        for kt in range(4):
            qeng[kt].dma_start(
                w_kts[kt][:], wqkv_ap.bitcast(mm_dt)[kt * 128 : (kt + 1) * 128, :]
            )
        nc.scalar.dma_start(
            wo_sb[:], wout_ap.bitcast(mm_dt).rearrange("(kt p) e -> p kt e", p=128)
        )
        nc.scalar.dma_start(b1[:], bout_ap.rearrange("(a d) -> a d", a=1))
        nc.gpsimd.partition_broadcast(bb[:], b1[:])

    def ld(ci):
        tb = ci * CHUNK

        # ---- load x chunk [128, subtile, D] (token-major) ----
        x_t = px.tile([128, SPC, D], F32, tag="x")
        for t in range(SPC):
            nc.sync.dma_start(
                x_t[:, t, :], x_ap[tb + t * 128 : tb + (t + 1) * 128, :]
            )

        # ---- transpose to xT: 4 tiles [128 dim, CHUNK tok] ----
        xts = []
        for db in range(4):
            xtp = pmm.tile([128, CHUNK], F32, tag="mm")
            for t in range(SPC):
                nc.tensor.transpose(
                    xtp[:, ts(t, 128)], x_t[:, t, ts(db, 128)], ident[:]
                )
            xt = pxt.tile([128, CHUNK], mm_dt, tag="xt", bufs=10)
            nc.any.tensor_copy(xt[:], xtp[:])
            xts.append(xt)
        return xts

    def qv(ci, xts):
        # ---- qT, kT in [feat, tok] layout: 8 ptiles of 128 feats ----
        qkd = {}
        qkod = {}
        # k-ptiles for quad 0 first: attention's first sim matmuls need
        # ptiles (4,5,0,1); emitting in that order shortens the wait
        for p in (4, 5, 0, 1, 6, 7, 2, 3):
            ps = pmm.tile([128, CHUNK], F32, tag="mm")
            for kt in range(4):
                nc.tensor.matmul(
                    ps[:],
                    w_kts[kt][:, ts(p, 128)],
                    xts[kt][:],
                    start=(kt == 0),
                    stop=(kt == 3),
                )
            qs = pqk.tile([128, CHUNK], at_dt, tag="qk", bufs=10)
            nc.vector.tensor_copy(qs[:], ps[:])
            qkd[p] = qs
            # odd heads live at partitions 64-127; matmul operands must sit
            # at base partition 0 (tile_position row 64 faults on this
            # runtime), so shift them down with SBUF->SBUF DMA right after
            # the cast (DMA is address-based)
            qo = pqk.tile([64, CHUNK], at_dt, tag="qko", name=f"qko{ci}_{p}", bufs=9)
            nc.sync.dma_start(qo[:], qs[64:128, :])
            qkod[p] = qo

        qkts = [qkd[p] for p in range(8)]
        qkos = [qkod[p] for p in range(8)]

        # ---- v natural [tok, feat] + ones column -> vext [128, h, 65] ----
        vexts = []
        for t in range(SPC):
            ps = pmm.tile([128, INNER], F32, tag="mm")
            for kt in range(4):
                nc.tensor.matmul(
                    ps[:],
                    xts[kt][:, ts(t, 128)],
                    w_kts[kt][:, 2 * INNER : 3 * INNER],
                    start=(kt == 0),
                    stop=(kt == 3),
                )
            vx = pvx.tile([128, HEADS, DH + 1], at_dt, tag="vx", bufs=5)
            nc.vector.memset(vx[:, :, DH : DH + 1].bitcast(F32), 1.0)
            nc.vector.tensor_copy(
                vx[:, :, 0:DH], ps[:].rearrange("p (h d) -> p h d", h=HEADS)
            )
            vexts.append(vx)

        return qkts, qkos, vexts

    def attn(ci, st):
        qkts, qkos, vexts = st
        # ---- attention, output written transposed into outT ptiles ----
        # Even heads (rows 0-63 of a ptile) write otls directly; odd heads
        # compute into base-0 tiles (oto) and are DMA-shifted to rows 64-127.
        otls = [
            pot.tile([128, CHUNK], mm_dt, tag="ot", name=f"ot{ci}_{i}")
            for i in range(4)
        ]
        otos = [
            pot.tile([64, CHUNK], mm_dt, tag="oto", name=f"oto{ci}_{i}")
            for i in range(4)
        ]
        half = []
        zst = prz.tile([97, 2 * FRAME], F32, tag="zst", name=f"zst{ci}_init")
        nc.vector.memset(zst[:], 1.0)
        for pr in range(4):  # head pairs (2*pr, 2*pr+1)
            # sim^T for both heads x both frames, per key-side 128-tok tile:
            # psum cols = (hp, fi) * FRAME
            pts = []
            for jt in range(2):
                sim = psim.tile([128, 4 * FRAME], F32, tag="sim")
                for hp in range(2):
                    h = 2 * pr + hp
                    if h % 2 == 0:
                        ck = qkts[4 + h // 2][0:64, :]
                        cq = qkts[h // 2][0:64, :]
                    else:
                        ck = qkos[4 + h // 2][:]
                        cq = qkos[h // 2][:]
                    for fi in range(FPC):
                        f0 = fi * FRAME
                        nc.tensor.matmul(
                            sim[:, ts(2 * hp + fi, FRAME)],
                            ck[:, f0 + jt * 128 : f0 + (jt + 1) * 128],
                            cq[:, f0 : f0 + FRAME],
                            start=True,
                            stop=True,
                        )
                pt = ppt.tile([128, 4 * FRAME], at_dt, tag="pt")
                nc.scalar.activation(pt[:], sim[:], AF.Exp, scale=SCALE)
                pts.append(pt)
            for fi in range(FPC):
                ovp = povp.tile([DH + 1, 2 * FRAME], F32, tag="ovp")
                for hp in range(2):
                    for jt in range(2):
                        nc.tensor.matmul(
                            ovp[:, ts(hp, FRAME)],
                            vexts[fi * 2 + jt][:, 2 * pr + hp, :],
                            pts[jt][:, ts(2 * hp + fi, FRAME)],
                            start=(jt == 0),
                            stop=(jt == 1),
                        )
                # unnormalized outputs (+ Z row) to SBUF; frees the PSUM tile
                ovs = pov.tile([DH + 1, 2 * FRAME], F32, tag="ovs", bufs=6)
                nc.vector.tensor_copy(ovs[:], ovp[:])
                # gather this pair's softmax denominators into the half-chunk
                # staging tile (DMA shifts partitions; DVE/ACT cannot)
                gi = 2 * (pr % 2) + fi
                nc.sync.dma_start(zst[32 * gi : 32 * gi + 1, :], ovs[DH : DH + 1, :])
                half.append((pr, fi, ovs))
            if pr % 2 == 1:
                # batch 1/Z = exp(-ln Z) for 4 (pair, frame) groups at once:
                # [4, 512] activations amortize the ~300-cycle ACT overhead
                lnst = prz.tile([97, 2 * FRAME], F32, tag="lnst")
                nc.scalar.activation(lnst[:], zst[:], AF.Ln)
                rzst = prz.tile([97, 2 * FRAME], F32, tag="rzst")
                nc.scalar.activation(rzst[:], lnst[:], AF.Exp, scale=-1.0)
                for bpr, bfi, ovs in half:
                    f0 = bfi * FRAME
                    gi = 2 * (bpr % 2) + bfi
                    # partition_broadcast reads garbage from a nonzero base
                    # partition on this runtime: DMA the row down to base 0
                    rz1 = prz.tile([1, 2 * FRAME], F32, tag="rz1", bufs=4)
                    nc.sync.dma_start(rz1[:], rzst[32 * gi : 32 * gi + 1, :])
                    rb = prb.tile([DH, 2 * FRAME], F32, tag="rb")
                    nc.gpsimd.partition_broadcast(rb[:], rz1[:])
                    for hp in range(2):
                        h = 2 * bpr + hp
                        dst = (
                            otls[h // 2][0:DH] if h % 2 == 0 else otos[h // 2][:]
                        )
                        nc.vector.tensor_mul(
                            dst[:, f0 : f0 + FRAME],
                            ovs[0:DH, ts(hp, FRAME)],
                            rb[:, ts(hp, FRAME)],
                        )
                    if bfi == FPC - 1:
                        # odd head of this pair is complete: shift its rows
                        # into the ptile so proj isn't gated on one big DMA
                        nc.sync.dma_start(otls[bpr][64:128, :], otos[bpr][:])
                half = []
                zst = prz.tile([97, 2 * FRAME], F32, tag="zst", name=f"zst{ci}_{pr}")
                nc.vector.memset(zst[:], 1.0)

        return otls

    def proj(ci, otls):
        tb = ci * CHUNK
        # ---- output projection + bias ----
        for s in range(SPC):
            ps = pmm.tile([128, D], F32, tag="mm")
            for p in range(4):
                nc.tensor.matmul(
                    ps[:],
                    otls[p][:, ts(s, 128)],
                    wo_sb[:, p, :],
                    start=(p == 0),
                    stop=(p == 3),
                )
            y = py.tile([128, D], F32, tag="y", bufs=3)
            nc.vector.scalar_tensor_tensor(
                y[:], ps[:], 1.0, bb[:], op0=ALU.mult, op1=ALU.add
            )
            nc.sync.dma_start(out_ap[tb + s * 128 : tb + (s + 1) * 128, :], y[:])

    # Software pipeline: emit the next chunk's transposes and qkv between
    # this chunk's attention and projection so PE has work while the
    # softmax-normalize chain (ACT->DVE->GPSIMD->DVE) drains; run the
    # transpose stage two chunks ahead so chunk 0's weight-load wait is
    # covered too.
    lds = {0: ld(0)}
    load_consts()
    if n_chunks > 1:
        lds[1] = ld(1)
    st = qv(0, lds.pop(0))
    for ci in range(n_chunks):
        otls = attn(ci, st)
        if ci + 1 < n_chunks:
            if ci + 2 < n_chunks:
                lds[ci + 2] = ld(ci + 2)
            st = qv(ci + 1, lds.pop(ci + 1))
        proj(ci, otls)


_CACHE = {}


def _get_nc(n_chunks=NCH):
    key = ("nc", n_chunks, str(MM_DT))
    if key in _CACHE:
        return _CACHE[key]
    from contextlib import ExitStack

    nc = bacc.Bacc("TRN2", target_bir_lowering=False, debug=False, num_devices=N_CORES)
    _pin_act_tables.arch = nc.m.arch
    _pin_act_tables()
    t_tok = n_chunks * CHUNK
    x_ap = nc.dram_tensor("x", [t_tok, D], F32, kind="ExternalInput").ap()
    wqkv_ap = nc.dram_tensor("w_qkv", [D, 3 * INNER], F32, kind="ExternalInput").ap()
    wout_ap = nc.dram_tensor("w_out", [INNER, D], F32, kind="ExternalInput").ap()
    bout_ap = nc.dram_tensor("b_out", [D], F32, kind="ExternalInput").ap()
    out_ap = nc.dram_tensor("out", [t_tok, D], F32, kind="ExternalOutput").ap()
    with tile.TileContext(nc) as tc:
        with ExitStack() as ctx:
            _build_body(
                nc, tc, ctx, x_ap, wqkv_ap, wout_ap, bout_ap, out_ap, n_chunks=n_chunks
            )
    nc.compile()
    _CACHE[key] = nc
    return nc


def _make_in_maps(x, w_qkv, w_out, b_out):
    x = np.ascontiguousarray(np.asarray(x, dtype=np.float32))
    w_qkv = np.ascontiguousarray(np.asarray(w_qkv, dtype=np.float32))
    w_out = np.ascontiguousarray(np.asarray(w_out, dtype=np.float32))
    b_out = np.ascontiguousarray(np.asarray(b_out, dtype=np.float32))
    assert x.shape == (B, SEQ, D), x.shape
    in_maps = []
    for c in range(N_CORES):
        b = c // 2
        t0 = (c % 2) * T
        in_maps.append(
            {
                "x": np.ascontiguousarray(x[b, t0 : t0 + T, :]),
                "w_qkv": w_qkv,
                "w_out": w_out,
                "b_out": b_out,
            }
        )
    return in_maps


def _assemble(results):
    out = np.empty((B, SEQ, D), dtype=np.float32)
    for c in range(N_CORES):
        b = c // 2
        t0 = (c % 2) * T
        out[b, t0 : t0 + T, :] = results[c]["out"]
    return out


def run(x, w_qkv, w_out, b_out, f=32, trace=False):
    assert int(f) == 32, f"kernel hardcoded for f=32, got {f}"
    _install_ntff_hook()
    nc = _get_nc()
    in_maps = _make_in_maps(x, w_qkv, w_out, b_out)
    res = run_bass_kernel_spmd(nc, in_maps, list(range(N_CORES)), trace=trace)
    return _assemble(res.results), res


def kernel(x, w_qkv, w_out, b_out, f=32):
    out, _ = run(x, w_qkv, w_out, b_out, f=f, trace=False)
    return out


# revision 53
# speedup vs baseline: 1.1834x; 1.1444x over previous
"""Axial (frame-local) attention kernel for Trainium2, 8-core data-parallel.

Problem: x[4, 8192, 512] -> qkv proj -> per-(batch, head, frame) attention over
256-token frames (f=32 frames of 256 tokens in an 8192 sequence) -> out proj.

Sharding: pure data-parallel over (batch, half-sequence): core c handles
batch c//2, tokens (c%2)*4096 .. +4096 (16 whole frames). No collectives.

Per-core pipeline (chunks of 512 tokens):
  - load x chunk, PE-transpose into xT [dim, tok] (feature-major)
  - qT,kT = (w_qkv block)^T-matmul in [feat, tok] layout; v natural [tok, feat]
  - per (frame, head): sim^T = k q^T on PE -> exp on ScalarE (no max-subtract;
    logits are O(6) so fp32 exp is safe) -> ov = [v|1]^T p~ on PE produces both
    the unnormalized attention output AND the softmax denominator Z (row 64)
  - normalize: 1/Z = exp(-ln Z) on ScalarE (DVE reciprocal is 8 cyc/elem —
    too slow), GPSIMD partition-broadcast, one DVE multiply
  - output projection from the transposed layout + bias, DMA out

Matmul operands use float32r (single-pass fp32, ~tf32 precision, 2x faster
than fp32's LOW_HIGH two-pass mode). PSUM accumulation stays fp32.
"""

import sys
import types

import numpy as np

import concourse.tile as tile
from concourse import bacc, mybir
from concourse.bass import ts
from concourse.bass_utils import run_bass_kernel_spmd
from concourse.masks import make_identity

F32 = mybir.dt.float32
F32R = mybir.dt.float32r
AF = mybir.ActivationFunctionType
ALU = mybir.AluOpType

# Model dims (hardcoded per problem spec)
B, SEQ, D = 4, 8192, 512
HEADS, DH = 8, 64
INNER = HEADS * DH  # 512
SCALE = DH ** -0.5
FRAME = 256  # n_sp = seq // f = 8192 // 32
N_CORES = 8
T = (B * SEQ) // N_CORES  # 4096 tokens per core
CHUNK = 512  # tokens per inner iteration
NCH = T // CHUNK  # 8
FPC = CHUNK // FRAME  # frames per chunk = 2
SPC = CHUNK // 128  # 128-token subtiles per chunk = 4

# matmul operand dtype: F32R (single-pass, ~tf32) or F32 (two-pass, exact)
MM_DT = F32R
# attention (sim / attn@v) operand dtype: BF16 enables fast-weight-load and
# LDWEIGHTS pull-ahead (f32r weight loads can't hide behind matmuls)
AT_DT = F32R

def _install_ntff_hook():
    """The trimmed container's antenv lacks axon_hooks; inject it so
    run_bass_kernel_spmd(trace=True) can capture NTFF profiles."""
    if "antenv.axon_hooks" in sys.modules:
        return
    try:
        from trn_agent_boot.trn_boot import _ntff_profile_via_ctypes

        hook = _ntff_profile_via_ctypes("/opt/axon/libaxon_pjrt.so")
    except Exception:
        return
    mod = types.ModuleType("antenv.axon_hooks")
    mod._hook = hook
    mod.get_axon_ntff_profile_hook = lambda: mod._hook
    mod.set_axon_ntff_profile_hook = lambda h: setattr(mod, "_hook", h)
    sys.modules["antenv.axon_hooks"] = mod


def _pin_act_tables():
    """Exp and Ln both live in the natural_log_exp_and_others table set, but
    the table-load chooser maps each function to the first set containing it,
    so alternating Exp/Ln activations reload tables (~1.3us each) every head
    pair. Restrict Exp/Ln to the combined set in the cached table map so one
    load covers the whole kernel."""
    from concourse.hw_specs import get_activation_tables

    tabs = get_activation_tables(_pin_act_tables.arch)
    keep = "natural_log_exp_and_others"
    if keep not in tabs:
        return
    for name, fns in tabs.items():
        if name != keep:
            fns.discard(AF.Exp)
            fns.discard(AF.Ln)


def _build_body(nc, tc, ctx, x_ap, wqkv_ap, wout_ap, bout_ap, out_ap, n_chunks=NCH):
    mm_dt = MM_DT
    at_dt = AT_DT
    pconst = ctx.enter_context(tc.tile_pool(name="const", bufs=1))
    px = ctx.enter_context(tc.tile_pool(name="x", bufs=2))
    pxt = ctx.enter_context(tc.tile_pool(name="xt", bufs=8))
    pqk = ctx.enter_context(tc.tile_pool(name="qk", bufs=16))
    pvx = ctx.enter_context(tc.tile_pool(name="vx", bufs=6))
    ppt = ctx.enter_context(tc.tile_pool(name="pt", bufs=4))
    prz = ctx.enter_context(tc.tile_pool(name="rz", bufs=2))
    prb = ctx.enter_context(tc.tile_pool(name="rb", bufs=3))
    pov = ctx.enter_context(tc.tile_pool(name="ovs", bufs=3))
    pot = ctx.enter_context(tc.tile_pool(name="ot", bufs=6))
    py = ctx.enter_context(tc.tile_pool(name="y", bufs=3))
    pmm = ctx.enter_context(tc.tile_pool(name="mm", bufs=2, space="PSUM"))
    psim = ctx.enter_context(tc.tile_pool(name="sim", bufs=2, space="PSUM"))
    povp = ctx.enter_context(tc.tile_pool(name="ovp", bufs=2, space="PSUM"))

    # Constants. The weight DMAs are emitted by load_consts() AFTER the
    # first two x-chunk loads so chunk 0's transposes aren't queued behind
    # 4MB of weights; spread across three DMA queues.
    ident = pconst.tile([128, 128], F32, tag="ident")
    make_identity(nc, ident[:])
    w_kts = [
        pconst.tile([128, 3 * INNER], mm_dt, tag=f"wqkv{kt}", name=f"wqkv{kt}")
        for kt in range(4)
    ]
    wo_sb = pconst.tile([128, 4, D], mm_dt, tag="wout")
    b1 = pconst.tile([1, D], F32, tag="b1")
    bb = pconst.tile([128, D], F32, tag="bb")

    def load_consts():
        qeng = [nc.scalar, nc.gpsimd, nc.scalar, nc.sync]
        for kt in range(4):
            qeng[kt].dma_start(
                w_kts[kt][:], wqkv_ap.bitcast(mm_dt)[kt * 128 : (kt + 1) * 128, :]
            )
        nc.scalar.dma_start(
            wo_sb[:], wout_ap.bitcast(mm_dt).rearrange("(kt p) e -> p kt e", p=128)
        )
        nc.scalar.dma_start(b1[:], bout_ap.rearrange("(a d) -> a d", a=1))
        nc.gpsimd.partition_broadcast(bb[:], b1[:])

    def ld(ci):
        tb = ci * CHUNK

        # ---- load x chunk [128, subtile, D] (token-major) ----
        x_t = px.tile([128, SPC, D], F32, tag="x")
        for t in range(SPC):
            nc.sync.dma_start(
                x_t[:, t, :], x_ap[tb + t * 128 : tb + (t + 1) * 128, :]
            )

        # ---- transpose to xT: 4 tiles [128 dim, CHUNK tok] ----
        xts = []
        for db in range(4):
            xtp = pmm.tile([128, CHUNK], F32, tag="mm")
            for t in range(SPC):
                nc.tensor.transpose(
                    xtp[:, ts(t, 128)], x_t[:, t, ts(db, 128)], ident[:]
                )
            xt = pxt.tile([128, CHUNK], mm_dt, tag="xt", bufs=10)
            nc.any.tensor_copy(xt[:], xtp[:])
            xts.append(xt)
        return xts

    def qv(ci, xts):
        # ---- qT, kT in [feat, tok] layout: 8 ptiles of 128 feats ----
        qkd = {}
        qkod = {}
        # k-ptiles for quad 0 first: attention's first sim matmuls need
        # ptiles (4,5,0,1); emitting in that order shortens the wait
        for p in (4, 5, 0, 1, 6, 7, 2, 3):
            ps = pmm.tile([128, CHUNK], F32, tag="mm")
            for kt in range(4):
                nc.tensor.matmul(
                    ps[:],
                    w_kts[kt][:, ts(p, 128)],
                    xts[kt][:],
                    start=(kt == 0),
                    stop=(kt == 3),
                )
            qs = pqk.tile([128, CHUNK], at_dt, tag="qk", bufs=10)
            nc.vector.tensor_copy(qs[:], ps[:])
            qkd[p] = qs
            # odd heads live at partitions 64-127; matmul operands must sit
            # at base partition 0 (tile_position row 64 faults on this
            # runtime), so shift them down with SBUF->SBUF DMA right after
            # the cast (DMA is address-based)
            qo = pqk.tile([64, CHUNK], at_dt, tag="qko", name=f"qko{ci}_{p}", bufs=9)
            nc.sync.dma_start(qo[:], qs[64:128, :])
            qkod[p] = qo

        qkts = [qkd[p] for p in range(8)]
        qkos = [qkod[p] for p in range(8)]

        # ---- v natural [tok, feat] + ones column -> vext [128, h, 65] ----
        vexts = []
        for t in range(SPC):
            ps = pmm.tile([128, INNER], F32, tag="mm")
            for kt in range(4):
                nc.tensor.matmul(
                    ps[:],
                    xts[kt][:, ts(t, 128)],
                    w_kts[kt][:, 2 * INNER : 3 * INNER],
                    start=(kt == 0),
                    stop=(kt == 3),
                )
            vx = pvx.tile([128, HEADS, DH + 1], at_dt, tag="vx", bufs=5)
            nc.vector.memset(vx[:, :, DH : DH + 1].bitcast(F32), 1.0)
            nc.vector.tensor_copy(
                vx[:, :, 0:DH], ps[:].rearrange("p (h d) -> p h d", h=HEADS)
            )
            vexts.append(vx)

        return qkts, qkos, vexts

    def attn(ci, st):
        qkts, qkos, vexts = st
        # ---- attention, output written transposed into outT ptiles ----
        # Even heads (rows 0-63 of a ptile) write otls directly; odd heads
        # compute into base-0 tiles (oto) and are DMA-shifted to rows 64-127.
        otls = [
            pot.tile([128, CHUNK], mm_dt, tag="ot", name=f"ot{ci}_{i}")
            for i in range(4)
        ]
        otos = [
            pot.tile([64, CHUNK], mm_dt, tag="oto", name=f"oto{ci}_{i}")
            for i in range(4)
        ]
        for pr in range(4):  # head pairs (2*pr, 2*pr+1)
            # sim^T for both heads x both frames, per key-side 128-tok tile:
            # psum cols = (hp, fi) * FRAME
            pts = []
            for jt in range(2):
                sim = psim.tile([128, 4 * FRAME], F32, tag="sim")
                for hp in range(2):
                    h = 2 * pr + hp
                    if h % 2 == 0:
                        ck = qkts[4 + h // 2][0:64, :]
                        cq = qkts[h // 2][0:64, :]
                    else:
                        ck = qkos[4 + h // 2][:]
                        cq = qkos[h // 2][:]
                    for fi in range(FPC):
                        f0 = fi * FRAME
                        nc.tensor.matmul(
                            sim[:, ts(2 * hp + fi, FRAME)],
                            ck[:, f0 + jt * 128 : f0 + (jt + 1) * 128],
                            cq[:, f0 : f0 + FRAME],
                            start=True,
                            stop=True,
                        )
                pt = ppt.tile([128, 4 * FRAME], at_dt, tag="pt")
                nc.scalar.activation(pt[:], sim[:], AF.Exp, scale=SCALE)
                pts.append(pt)
            for fi in range(FPC):
                f0 = fi * FRAME
                ovp = povp.tile([DH + 1, 2 * FRAME], F32, tag="ovp")
                for hp in range(2):
                    for jt in range(2):
                        nc.tensor.matmul(
                            ovp[:, ts(hp, FRAME)],
                            vexts[fi * 2 + jt][:, 2 * pr + hp, :],
                            pts[jt][:, ts(2 * hp + fi, FRAME)],
                            start=(jt == 0),
                            stop=(jt == 1),
                        )
                # softmax denominators for both heads: rz = exp(-ln Z)
                lnz = prz.tile([1, 2 * FRAME], F32, tag="lnz", bufs=3)
                nc.scalar.activation(lnz[:], ovp[DH : DH + 1, :], AF.Ln)
                # unnormalized outputs to SBUF; frees the PSUM tile
                ovs = pov.tile([DH, 2 * FRAME], F32, tag="ovs")
                nc.vector.tensor_copy(ovs[:], ovp[0:DH, :])
                rz = prz.tile([1, 2 * FRAME], F32, tag="rz", bufs=3)
                nc.scalar.activation(rz[:], lnz[:], AF.Exp, scale=-1.0)
                rb = prb.tile([DH, 2 * FRAME], F32, tag="rb")
                nc.gpsimd.partition_broadcast(rb[:], rz[:])
                for hp in range(2):
                    h = 2 * pr + hp
                    dst = otls[h // 2][0:DH] if h % 2 == 0 else otos[h // 2][:]
                    nc.vector.tensor_mul(
                        dst[:, f0 : f0 + FRAME],
                        ovs[:, ts(hp, FRAME)],
                        rb[:, ts(hp, FRAME)],
                    )
                if fi == FPC - 1:
                    # odd head of this pair is complete: shift its rows into
                    # the ptile now so proj isn't gated on one big DMA
                    nc.sync.dma_start(otls[pr][64:128, :], otos[pr][:])

        return otls

    def proj(ci, otls):
        tb = ci * CHUNK
        # ---- output projection + bias ----
        for s in range(SPC):
            ps = pmm.tile([128, D], F32, tag="mm")
            for p in range(4):
                nc.tensor.matmul(
                    ps[:],
                    otls[p][:, ts(s, 128)],
                    wo_sb[:, p, :],
                    start=(p == 0),
                    stop=(p == 3),
                )
            y = py.tile([128, D], F32, tag="y", bufs=3)
            nc.vector.scalar_tensor_tensor(
                y[:], ps[:], 1.0, bb[:], op0=ALU.mult, op1=ALU.add
            )
            nc.sync.dma_start(out_ap[tb + s * 128 : tb + (s + 1) * 128, :], y[:])

    # Software pipeline: emit the next chunk's transposes and qkv between
    # this chunk's attention and projection so PE has work while the
    # softmax-normalize chain (ACT->DVE->GPSIMD->DVE) drains; run the
    # transpose stage two chunks ahead so chunk 0's weight-load wait is
    # covered too.
    lds = {0: ld(0)}
    load_consts()
    if n_chunks > 1:
        lds[1] = ld(1)
    st = qv(0, lds.pop(0))
    for ci in range(n_chunks):
        otls = attn(ci, st)
        if ci + 1 < n_chunks:
            if ci + 2 < n_chunks:
                lds[ci + 2] = ld(ci + 2)
            st = qv(ci + 1, lds.pop(ci + 1))
        proj(ci, otls)


_CACHE = {}


def _get_nc(n_chunks=NCH):
    key = ("nc", n_chunks, str(MM_DT))
    if key in _CACHE:
        return _CACHE[key]
    from contextlib import ExitStack

    nc = bacc.Bacc("TRN2", target_bir_lowering=False, debug=False, num_devices=N_CORES)
    _pin_act_tables.arch = nc.m.arch
    _pin_act_tables()
    t_tok = n_chunks * CHUNK
    x_ap = nc.dram_tensor("x", [t_tok, D], F32, kind="ExternalInput").ap()
    wqkv_ap = nc.dram_tensor("w_qkv", [D, 3 * INNER], F32, kind="ExternalInput").ap()
    wout_ap = nc.dram_tensor("w_out", [INNER, D], F32, kind="ExternalInput").ap()
    bout_ap = nc.dram_tensor("b_out", [D], F32, kind="ExternalInput").ap()
    out_ap = nc.dram_tensor("out", [t_tok, D], F32, kind="ExternalOutput").ap()
    with tile.TileContext(nc) as tc:
        with ExitStack() as ctx:
            _build_body(
                nc, tc, ctx, x_ap, wqkv_ap, wout_ap, bout_ap, out_ap, n_chunks=n_chunks
            )
    nc.compile()
    _CACHE[key] = nc
    return nc


def _make_in_maps(x, w_qkv, w_out, b_out):
    x = np.ascontiguousarray(np.asarray(x, dtype=np.float32))
    w_qkv = np.ascontiguousarray(np.asarray(w_qkv, dtype=np.float32))
    w_out = np.ascontiguousarray(np.asarray(w_out, dtype=np.float32))
    b_out = np.ascontiguousarray(np.asarray(b_out, dtype=np.float32))
    assert x.shape == (B, SEQ, D), x.shape
    in_maps = []
    for c in range(N_CORES):
        b = c // 2
        t0 = (c % 2) * T
        in_maps.append(
            {
                "x": np.ascontiguousarray(x[b, t0 : t0 + T, :]),
                "w_qkv": w_qkv,
                "w_out": w_out,
                "b_out": b_out,
            }
        )
    return in_maps


def _assemble(results):
    out = np.empty((B, SEQ, D), dtype=np.float32)
    for c in range(N_CORES):
        b = c // 2
        t0 = (c % 2) * T
        out[b, t0 : t0 + T, :] = results[c]["out"]
    return out


def run(x, w_qkv, w_out, b_out, f=32, trace=False):
    assert int(f) == 32, f"kernel hardcoded for f=32, got {f}"
    _install_ntff_hook()
    nc = _get_nc()
    in_maps = _make_in_maps(x, w_qkv, w_out, b_out)
    res = run_bass_kernel_spmd(nc, in_maps, list(range(N_CORES)), trace=trace)
    return _assemble(res.results), res


def kernel(x, w_qkv, w_out, b_out, f=32):
    out, _ = run(x, w_qkv, w_out, b_out, f=f, trace=False)
    return out


# revision 56
# speedup vs baseline: 1.2131x; 1.0251x over previous
"""Axial (frame-local) attention kernel for Trainium2, 8-core data-parallel.

Problem: x[4, 8192, 512] -> qkv proj -> per-(batch, head, frame) attention over
256-token frames (f=32 frames of 256 tokens in an 8192 sequence) -> out proj.

Sharding: pure data-parallel over (batch, half-sequence): core c handles
batch c//2, tokens (c%2)*4096 .. +4096 (16 whole frames). No collectives.

Per-core pipeline (chunks of 512 tokens):
  - load x chunk, PE-transpose into xT [dim, tok] (feature-major)
  - qT,kT = (w_qkv block)^T-matmul in [feat, tok] layout; v natural [tok, feat]
  - per (frame, head): sim^T = k q^T on PE -> exp on ScalarE (no max-subtract;
    logits are O(6) so fp32 exp is safe) -> ov = [v|1]^T p~ on PE produces both
    the unnormalized attention output AND the softmax denominator Z (row 64)
  - normalize: 1/Z = exp(-ln Z) on ScalarE (DVE reciprocal is 8 cyc/elem —
    too slow), GPSIMD partition-broadcast, one DVE multiply
  - output projection from the transposed layout + bias, DMA out

Matmul operands use float32r (single-pass fp32, ~tf32 precision, 2x faster
than fp32's LOW_HIGH two-pass mode). PSUM accumulation stays fp32.
"""

import sys
import types

import numpy as np

import concourse.tile as tile
from concourse import bacc, mybir
from concourse.bass import ts
from concourse.bass_utils import run_bass_kernel_spmd
from concourse.masks import make_identity

F32 = mybir.dt.float32
F32R = mybir.dt.float32r
AF = mybir.ActivationFunctionType
ALU = mybir.AluOpType

# Model dims (hardcoded per problem spec)
B, SEQ, D = 4, 8192, 512
HEADS, DH = 8, 64
INNER = HEADS * DH  # 512
SCALE = DH ** -0.5
FRAME = 256  # n_sp = seq // f = 8192 // 32
N_CORES = 8
T = (B * SEQ) // N_CORES  # 4096 tokens per core
CHUNK = 512  # tokens per inner iteration
NCH = T // CHUNK  # 8
FPC = CHUNK // FRAME  # frames per chunk = 2
SPC = CHUNK // 128  # 128-token subtiles per chunk = 4

# matmul operand dtype: F32R (single-pass, ~tf32) or F32 (two-pass, exact)
MM_DT = F32R
# attention (sim / attn@v) operand dtype: BF16 enables fast-weight-load and
# LDWEIGHTS pull-ahead (f32r weight loads can't hide behind matmuls)
AT_DT = F32R

def _install_ntff_hook():
    """The trimmed container's antenv lacks axon_hooks; inject it so
    run_bass_kernel_spmd(trace=True) can capture NTFF profiles."""
    if "antenv.axon_hooks" in sys.modules:
        return
    try:
        from trn_agent_boot.trn_boot import _ntff_profile_via_ctypes

        hook = _ntff_profile_via_ctypes("/opt/axon/libaxon_pjrt.so")
    except Exception:
        return
    mod = types.ModuleType("antenv.axon_hooks")
    mod._hook = hook
    mod.get_axon_ntff_profile_hook = lambda: mod._hook
    mod.set_axon_ntff_profile_hook = lambda h: setattr(mod, "_hook", h)
    sys.modules["antenv.axon_hooks"] = mod


def _pin_act_tables():
    """Exp and Ln both live in the natural_log_exp_and_others table set, but
    the table-load chooser maps each function to the first set containing it,
    so alternating Exp/Ln activations reload tables (~1.3us each) every head
    pair. Restrict Exp/Ln to the combined set in the cached table map so one
    load covers the whole kernel."""
    from concourse.hw_specs import get_activation_tables

    tabs = get_activation_tables(_pin_act_tables.arch)
    keep = "natural_log_exp_and_others"
    if keep not in tabs:
        return
    for name, fns in tabs.items():
        if name != keep:
            fns.discard(AF.Exp)
            fns.discard(AF.Ln)


def _build_body(nc, tc, ctx, x_ap, wqkv_ap, wout_ap, bout_ap, out_ap, n_chunks=NCH):
    mm_dt = MM_DT
    at_dt = AT_DT
    pconst = ctx.enter_context(tc.tile_pool(name="const", bufs=1))
    px = ctx.enter_context(tc.tile_pool(name="x", bufs=2))
    pxt = ctx.enter_context(tc.tile_pool(name="xt", bufs=8))
    pqk = ctx.enter_context(tc.tile_pool(name="qk", bufs=16))
    pvx = ctx.enter_context(tc.tile_pool(name="vx", bufs=6))
    ppt = ctx.enter_context(tc.tile_pool(name="pt", bufs=4))
    prz = ctx.enter_context(tc.tile_pool(name="rz", bufs=2))
    prb = ctx.enter_context(tc.tile_pool(name="rb", bufs=3))
    pov = ctx.enter_context(tc.tile_pool(name="ovs", bufs=3))
    pot = ctx.enter_context(tc.tile_pool(name="ot", bufs=6))
    py = ctx.enter_context(tc.tile_pool(name="y", bufs=3))
    pmm = ctx.enter_context(tc.tile_pool(name="mm", bufs=2, space="PSUM"))
    psim = ctx.enter_context(tc.tile_pool(name="sim", bufs=2, space="PSUM"))
    povp = ctx.enter_context(tc.tile_pool(name="ovp", bufs=2, space="PSUM"))

    # Constants. The weight DMAs are emitted by load_consts() AFTER the
    # first two x-chunk loads so chunk 0's transposes aren't queued behind
    # 4MB of weights; spread across three DMA queues.
    ident = pconst.tile([128, 128], F32, tag="ident")
    make_identity(nc, ident[:])
    w_kts = [
        pconst.tile([128, 3 * INNER], mm_dt, tag=f"wqkv{kt}", name=f"wqkv{kt}")
        for kt in range(4)
    ]
    wo_sb = pconst.tile([128, 4, D], mm_dt, tag="wout")
    b1 = pconst.tile([1, D], F32, tag="b1")
    bb = pconst.tile([128, D], F32, tag="bb")

    def load_consts():
        qeng = [nc.scalar, nc.gpsimd, nc.scalar, nc.sync]
        for kt in range(4):
            qeng[kt].dma_start(
                w_kts[kt][:], wqkv_ap.bitcast(mm_dt)[kt * 128 : (kt + 1) * 128, :]
            )
        nc.scalar.dma_start(
            wo_sb[:], wout_ap.bitcast(mm_dt).rearrange("(kt p) e -> p kt e", p=128)
        )
        nc.scalar.dma_start(b1[:], bout_ap.rearrange("(a d) -> a d", a=1))
        nc.gpsimd.partition_broadcast(bb[:], b1[:])

    def ld(ci):
        tb = ci * CHUNK

        # ---- load x chunk [128, subtile, D] (token-major) ----
        x_t = px.tile([128, SPC, D], F32, tag="x")
        for t in range(SPC):
            nc.sync.dma_start(
                x_t[:, t, :], x_ap[tb + t * 128 : tb + (t + 1) * 128, :]
            )

        # ---- transpose to xT: 4 tiles [128 dim, CHUNK tok] ----
        xts = []
        for db in range(4):
            xtp = pmm.tile([128, CHUNK], F32, tag="mm")
            for t in range(SPC):
                nc.tensor.transpose(
                    xtp[:, ts(t, 128)], x_t[:, t, ts(db, 128)], ident[:]
                )
            xt = pxt.tile([128, CHUNK], mm_dt, tag="xt", bufs=10)
            nc.any.tensor_copy(xt[:], xtp[:])
            xts.append(xt)
        return xts

    def qv(ci, xts):
        # ---- qT, kT in [feat, tok] layout: 8 ptiles of 128 feats ----
        qkd = {}
        qkod = {}
        # k-ptiles for quad 0 first: attention's first sim matmuls need
        # ptiles (4,5,0,1); emitting in that order shortens the wait
        for p in (4, 5, 0, 1, 6, 7, 2, 3):
            ps = pmm.tile([128, CHUNK], F32, tag="mm")
            for kt in range(4):
                nc.tensor.matmul(
                    ps[:],
                    w_kts[kt][:, ts(p, 128)],
                    xts[kt][:],
                    start=(kt == 0),
                    stop=(kt == 3),
                )
            qs = pqk.tile([128, CHUNK], at_dt, tag="qk", bufs=10)
            nc.vector.tensor_copy(qs[:], ps[:])
            qkd[p] = qs
            # odd heads live at partitions 64-127; matmul operands must sit
            # at base partition 0 (tile_position row 64 faults on this
            # runtime), so shift them down with SBUF->SBUF DMA right after
            # the cast (DMA is address-based)
            qo = pqk.tile([64, CHUNK], at_dt, tag="qko", name=f"qko{ci}_{p}", bufs=9)
            nc.sync.dma_start(qo[:], qs[64:128, :])
            qkod[p] = qo

        qkts = [qkd[p] for p in range(8)]
        qkos = [qkod[p] for p in range(8)]

        # ---- v natural [tok, feat] + ones column -> vext [128, h, 65] ----
        vexts = []
        for t in range(SPC):
            ps = pmm.tile([128, INNER], F32, tag="mm")
            for kt in range(4):
                nc.tensor.matmul(
                    ps[:],
                    xts[kt][:, ts(t, 128)],
                    w_kts[kt][:, 2 * INNER : 3 * INNER],
                    start=(kt == 0),
                    stop=(kt == 3),
                )
            vx = pvx.tile([128, HEADS, DH + 1], at_dt, tag="vx", bufs=5)
            nc.vector.memset(vx[:, :, DH : DH + 1].bitcast(F32), 1.0)
            nc.vector.tensor_copy(
                vx[:, :, 0:DH], ps[:].rearrange("p (h d) -> p h d", h=HEADS)
            )
            vexts.append(vx)

        return qkts, qkos, vexts

    def attn(ci, st):
        qkts, qkos, vexts = st
        # ---- attention, output written transposed into outT ptiles ----
        # Even heads (rows 0-63 of a ptile) write otls directly; odd heads
        # compute into base-0 tiles (oto) and are DMA-shifted to rows 64-127.
        otls = [
            pot.tile([128, CHUNK], mm_dt, tag="ot", name=f"ot{ci}_{i}")
            for i in range(4)
        ]
        otos = [
            pot.tile([64, CHUNK], mm_dt, tag="oto", name=f"oto{ci}_{i}")
            for i in range(4)
        ]
        for pr in range(4):  # head pairs (2*pr, 2*pr+1)
            # sim^T for both heads x both frames, per key-side 128-tok tile:
            # psum cols = (hp, fi) * FRAME
            pts = []
            for jt in range(2):
                sim = psim.tile([128, 4 * FRAME], F32, tag="sim")
                for hp in range(2):
                    h = 2 * pr + hp
                    if h % 2 == 0:
                        ck = qkts[4 + h // 2][0:64, :]
                        cq = qkts[h // 2][0:64, :]
                    else:
                        ck = qkos[4 + h // 2][:]
                        cq = qkos[h // 2][:]
                    for fi in range(FPC):
                        f0 = fi * FRAME
                        nc.tensor.matmul(
                            sim[:, ts(2 * hp + fi, FRAME)],
                            ck[:, f0 + jt * 128 : f0 + (jt + 1) * 128],
                            cq[:, f0 : f0 + FRAME],
                            start=True,
                            stop=True,
                        )
                pt = ppt.tile([128, 4 * FRAME], at_dt, tag="pt")
                nc.scalar.activation(pt[:], sim[:], AF.Exp, scale=SCALE)
                pts.append(pt)
            for fi in range(FPC):
                f0 = fi * FRAME
                ovp = povp.tile([DH + 1, 2 * FRAME], F32, tag="ovp")
                for hp in range(2):
                    for jt in range(2):
                        nc.tensor.matmul(
                            ovp[:, ts(hp, FRAME)],
                            vexts[fi * 2 + jt][:, 2 * pr + hp, :],
                            pts[jt][:, ts(2 * hp + fi, FRAME)],
                            start=(jt == 0),
                            stop=(jt == 1),
                        )
                # softmax denominators for both heads: rz = exp(-ln Z)
                lnz = prz.tile([1, 2 * FRAME], F32, tag="lnz", bufs=3)
                nc.scalar.activation(lnz[:], ovp[DH : DH + 1, :], AF.Ln)
                rz = prz.tile([1, 2 * FRAME], F32, tag="rz", bufs=3)
                nc.scalar.activation(rz[:], lnz[:], AF.Exp, scale=-1.0)
                # unnormalized outputs to SBUF on ScalarE; frees the PSUM
                # tile and keeps DVE (the per-chunk co-bottleneck) lighter
                ovs = pov.tile([DH, 2 * FRAME], F32, tag="ovs")
                nc.scalar.copy(ovs[:], ovp[0:DH, :])
                rb = prb.tile([DH, 2 * FRAME], F32, tag="rb")
                nc.gpsimd.partition_broadcast(rb[:], rz[:])
                for hp in range(2):
                    h = 2 * pr + hp
                    dst = otls[h // 2][0:DH] if h % 2 == 0 else otos[h // 2][:]
                    nc.vector.tensor_mul(
                        dst[:, f0 : f0 + FRAME],
                        ovs[:, ts(hp, FRAME)],
                        rb[:, ts(hp, FRAME)],
                    )
                if fi == FPC - 1:
                    # odd head of this pair is complete: shift its rows into
                    # the ptile now so proj isn't gated on one big DMA
                    nc.sync.dma_start(otls[pr][64:128, :], otos[pr][:])

        return otls

    def proj(ci, otls):
        tb = ci * CHUNK
        # ---- output projection + bias ----
        for s in range(SPC):
            ps = pmm.tile([128, D], F32, tag="mm")
            for p in range(4):
                nc.tensor.matmul(
                    ps[:],
                    otls[p][:, ts(s, 128)],
                    wo_sb[:, p, :],
                    start=(p == 0),
                    stop=(p == 3),
                )
            y = py.tile([128, D], F32, tag="y", bufs=3)
            nc.vector.scalar_tensor_tensor(
                y[:], ps[:], 1.0, bb[:], op0=ALU.mult, op1=ALU.add
            )
            nc.sync.dma_start(out_ap[tb + s * 128 : tb + (s + 1) * 128, :], y[:])

    # Software pipeline: emit the next chunk's transposes and qkv between
    # this chunk's attention and projection so PE has work while the
    # softmax-normalize chain (ACT->DVE->GPSIMD->DVE) drains; run the
    # transpose stage two chunks ahead so chunk 0's weight-load wait is
    # covered too.
    lds = {0: ld(0)}
    load_consts()
    if n_chunks > 1:
        lds[1] = ld(1)
    st = qv(0, lds.pop(0))
    for ci in range(n_chunks):
        otls = attn(ci, st)
        if ci + 1 < n_chunks:
            if ci + 2 < n_chunks:
                lds[ci + 2] = ld(ci + 2)
            st = qv(ci + 1, lds.pop(ci + 1))
        proj(ci, otls)


_CACHE = {}


def _get_nc(n_chunks=NCH):
    key = ("nc", n_chunks, str(MM_DT))
    if key in _CACHE:
        return _CACHE[key]
    from contextlib import ExitStack

    nc = bacc.Bacc("TRN2", target_bir_lowering=False, debug=False, num_devices=N_CORES)
    _pin_act_tables.arch = nc.m.arch
    _pin_act_tables()
    t_tok = n_chunks * CHUNK
    x_ap = nc.dram_tensor("x", [t_tok, D], F32, kind="ExternalInput").ap()
    wqkv_ap = nc.dram_tensor("w_qkv", [D, 3 * INNER], F32, kind="ExternalInput").ap()
    wout_ap = nc.dram_tensor("w_out", [INNER, D], F32, kind="ExternalInput").ap()
    bout_ap = nc.dram_tensor("b_out", [D], F32, kind="ExternalInput").ap()
    out_ap = nc.dram_tensor("out", [t_tok, D], F32, kind="ExternalOutput").ap()
    with tile.TileContext(nc) as tc:
        with ExitStack() as ctx:
            _build_body(
                nc, tc, ctx, x_ap, wqkv_ap, wout_ap, bout_ap, out_ap, n_chunks=n_chunks
            )
    nc.compile()
    _CACHE[key] = nc
    return nc


def _make_in_maps(x, w_qkv, w_out, b_out):
    x = np.ascontiguousarray(np.asarray(x, dtype=np.float32))
    w_qkv = np.ascontiguousarray(np.asarray(w_qkv, dtype=np.float32))
    w_out = np.ascontiguousarray(np.asarray(w_out, dtype=np.float32))
    b_out = np.ascontiguousarray(np.asarray(b_out, dtype=np.float32))
    assert x.shape == (B, SEQ, D), x.shape
    in_maps = []
    for c in range(N_CORES):
        b = c // 2
        t0 = (c % 2) * T
        in_maps.append(
            {
                "x": np.ascontiguousarray(x[b, t0 : t0 + T, :]),
                "w_qkv": w_qkv,
                "w_out": w_out,
                "b_out": b_out,
            }
        )
    return in_maps


def _assemble(results):
    out = np.empty((B, SEQ, D), dtype=np.float32)
    for c in range(N_CORES):
        b = c // 2
        t0 = (c % 2) * T
        out[b, t0 : t0 + T, :] = results[c]["out"]
    return out


def run(x, w_qkv, w_out, b_out, f=32, trace=False):
    assert int(f) == 32, f"kernel hardcoded for f=32, got {f}"
    _install_ntff_hook()
    nc = _get_nc()
    in_maps = _make_in_maps(x, w_qkv, w_out, b_out)
    res = run_bass_kernel_spmd(nc, in_maps, list(range(N_CORES)), trace=trace)
    return _assemble(res.results), res


def kernel(x, w_qkv, w_out, b_out, f=32):
    out, _ = run(x, w_qkv, w_out, b_out, f=f, trace=False)
    return out
